# revision 30
# baseline (speedup 1.0000x reference)
"""Trainium2 Bass kernel for nn_Eva_62818191671438 (sparse_attention).

Sharding: 8 cores = (batch b in 0..3) x (head-half in 0..1); each core runs
QKV projection + attention for its 8 heads + partial output projection
(row-parallel TP); host sums the two partials per batch and adds proj_b.

Structural optimizations:
- The T=2 per-head-token attentions share the big S x S spatial
  logits/exp/AV work; only rank-1 corrections differ per token.
- Softmax uses plain exp (logits ~N(0,1): no overflow risk) with the
  denominator obtained free via a ones-column appended to V.
- RoPE's interleaved rotation becomes two contiguous partition-block
  multiplies after an evens-then-odds permutation of each head's D axis
  (folded into the QKV weights host-side); runs on DVE in bf16 2x/4x mode
  from a gpsimd-evacuated bf16 copy of the QKV psum.
- Engine balance: PE does all matmuls (f32r/bf16 single pass), ACT does
  exp, gpsimd does psum evacuations, DVE does rope + softmax corrections;
  input DMAs are split across the SP and ACT hwdge queues.
- Softmax corrections computed on [heads, N] tiles; token/denominator
  rows land there via 3 small DMAs per head (no transpose gathers).
"""
import sys
sys.path.insert(0, "/opt/trn_rl_repo")
import numpy as np

B, N, C, H, T, D, S = 4, 1026, 1024, 16, 2, 64, 1024
HPC = 8          # heads per core
NCORES = 8
KC = 8           # contraction chunks (C/128)
QCW = 344        # attention query-chunk width cap (8B-aligned starts)
NP = 1028        # padded inner stride for bf16 rhs tiles
QCH = [(0, 344), (344, 688), (688, 1026)]
SCALE = D ** -0.5

_CACHE = {}


def _build_nc(shuffle_engine="vector", debug=False, dump=False, repeat=1,
              serial=False):
    import concourse.bacc as bacc
    import concourse.mybir as mybir
    import concourse.tile as tile

    F32 = mybir.dt.float32
    F32R = mybir.dt.float32r
    MDT = mybir.dt.bfloat16
    AF = mybir.ActivationFunctionType
    MUL = mybir.AluOpType.mult
    ADD = mybir.AluOpType.add

    nc = bacc.Bacc("TRN2", target_bir_lowering=False, debug=debug,
                   num_devices=NCORES)
    xT = nc.dram_tensor("xT", [1025, N], F32R, kind="ExternalInput").ap()
    wT = nc.dram_tensor("wT", [1025, 1536], F32R, kind="ExternalInput").ap()
    pjT = nc.dram_tensor("pjT", [512, 1024], F32R, kind="ExternalInput").ap()
    cosP = nc.dram_tensor("cosP", [128, S], F32, kind="ExternalInput").ap()
    sinAF = nc.dram_tensor("sinAF", [128, S], F32, kind="ExternalInput").ap()
    outs = [nc.dram_tensor("out" if r == 0 else f"out{r}", [N, C], F32,
                            kind="ExternalOutput").ap() for r in range(repeat)]

    dbg = {}
    if dump:
        for nm, shp in [("d_qT", [128, 4, N]), ("d_kT", [128, 4, N]),
                        ("d_avsb", [65, HPC, N]), ("d_zb", [16, N]),
                        ("d_sgs", [8, N]), ("d_sgg0", [8, N]),
                        ("d_oT", [128, 4, N]), ("d_vz", [128, KC, HPC, 65])]:
            dt_ = F32R if nm == "d_oT" else MDT
            dbg[nm] = nc.dram_tensor(nm, shp, dt_, kind="ExternalOutput").ap()
    with tile.TileContext(nc) as tc:
        for rep in range(repeat):
            _emit(nc, tc, tile, mybir, F32, MDT, AF, MUL, ADD,
                  xT, wT, pjT, cosP, sinAF, outs[rep],
                  shuffle_engine,
                  dbg if rep == 0 else {}, sfx=str(rep) if repeat > 1 else "",
                  chain=(outs[rep - 1] if (serial and rep > 0) else None))
    nc.compile()
    return nc


def _emit(nc, tc, tile, mybir, F32, MDT, AF, MUL, ADD,
          xT, wT, pjT, cosP, sinAF, out, shuffle_engine,
          dbg={}, sfx="", chain=None):
    F32R = mybir.dt.float32r
    from contextlib import ExitStack
    ctx = ExitStack()
    with ctx:
        pm = ctx.enter_context(tc.tile_pool(name="pm" + sfx, bufs=1))
        psum = ctx.enter_context(tc.tile_pool(name="psum" + sfx, bufs=1, space="PSUM"))

        # ---------------- persistent tiles ----------------
        qT = pm.tile([128, 4, N], F32R, name="qT")
        kT = pm.tile([128, 4, N], F32R, name="kT")
        vz = pm.tile([128, KC, HPC, 65], MDT, name="vz")
        vtok0 = pm.tile([1, HPC, 65], F32R, name="vtok0")
        vtok1 = pm.tile([1, HPC, 65], F32R, name="vtok1")
        oT = pm.tile([128, 4, N], F32R, name="oT")
        pjts = pm.tile([128, 4, 1024], F32R, name="pjts")
        ones64 = pm.tile([128, 64], F32, name="ones64")
        if chain is not None:
            chaint = pm.tile([1, C], F32, name="chaint")
            nc.sync.dma_start(chaint[:], chain[0:1, :])
            nc.vector.tensor_copy(ones64[0:1, 0:1], chaint[0:1, 0:1])
        nc.gpsimd.memset(ones64[:], 1.0)
        nc.vector.tensor_copy(
            vz[:, :, :, 64:65],
            ones64[:].rearrange("p (a b o) -> p a b o", a=KC, b=HPC))


        # ---------------- stage A+B: load + QKV ----------------
        with tc.tile_pool(name="pw" + sfx, bufs=1) as pw, \
             tc.tile_pool(name="pr" + sfx, bufs=3) as pr:
            xts = pw.tile([128, KC, N], F32R, name="xts")
            xone = pw.tile([1, N], F32R, name="xone")
            wts = pw.tile([128, KC, 1536], F32R, name="wts")
            wb = pw.tile([1, 1536], F32R, name="wb")
            cosPt = pw.tile([128, S], F32, name="cosPt")
            sinFt = pw.tile([128, S], F32, name="sinFt")
            # first contraction chunks lead so the PE starts ASAP; the
            # bias/rope rows ride between chunk issues (needed later);
            # issue is split across the SP and ACT hwdge queues
            for kc in range(KC):
                qw_ = nc.sync if kc % 2 == 0 else nc.scalar
                qx_ = nc.scalar if kc % 2 == 0 else nc.sync
                qw_.dma_start(wts[:, kc], wT[128 * kc:128 * (kc + 1)])
                qx_.dma_start(xts[:, kc], xT[128 * kc:128 * (kc + 1)])
                if kc == 1:
                    nc.sync.dma_start(wb[:], wT[1024:1025])
                    nc.scalar.dma_start(xone[:], xT[1024:1025])
                elif kc == 3:
                    nc.sync.dma_start(cosPt[:], cosP[:])
                    nc.scalar.dma_start(sinFt[:], sinAF[:])
            for j in range(4):
                nc.scalar.dma_start(pjts[:, j], pjT[128 * j:128 * (j + 1)])

            # q/k groups: out [dout 128, n-chunk]; rope on spatial chunks
            FQK = [(0, 2), (2, 514), (514, 1026)]
            for qk in range(2):          # 0=q, 1=k
                dst = qT if qk == 0 else kT
                for g in range(4):
                    gc = qk * 512 + g * 128
                    for (f0, f1) in FQK:
                        fw = f1 - f0
                        ps = psum.tile([128, 512], F32, tag="av", name="psqk", bufs=3)
                        for kc in range(KC):
                            nc.tensor.matmul(
                                ps[:, :fw], wts[:, kc, gc:gc + 128],
                                xts[:, kc, f0:f1], start=(kc == 0), stop=False)
                        nc.tensor.matmul(
                            ps[:, :fw], wb[:, gc:gc + 128], xone[:, f0:f1],
                            start=False, stop=True)
                        if f0 == 0:      # token cols: no rope, plain copy
                            nc.scalar.activation(dst[:, g, 0:2], ps[:, :2],
                                                 AF.Identity)
                            continue
                        sl = slice(f0 - 2, f1 - 2)
                        tmp = pr.tile([128, 512], F32, tag="rtmp", name="rtmp")
                        cq = pr.tile([128, 512], F32, tag="rcq", name="rcq")
                        for hb in (0, 64):   # two heads per chunk
                            nc.vector.tensor_tensor(
                                out=tmp[hb:hb + 32, :fw], in0=ps[hb + 32:hb + 64, :fw],
                                in1=sinFt[hb:hb + 32, sl], op=MUL)
                            nc.vector.tensor_tensor(
                                out=tmp[hb + 32:hb + 64, :fw], in0=ps[hb:hb + 32, :fw],
                                in1=sinFt[hb + 32:hb + 64, sl], op=MUL)
                        nc.vector.tensor_tensor(
                            out=cq[:, :fw], in0=ps[:, :fw], in1=cosPt[:, sl], op=MUL)
                        nc.vector.tensor_tensor(
                            out=dst[:, g, f0:f1], in0=cq[:, :fw], in1=tmp[:, :fw],
                            op=ADD)

            # v groups: out [n-chunk, dout 512]
            NSL = [(0, 2)] + [(2 + 128 * i, 2 + 128 * (i + 1)) for i in range(8)]
            for si, (n0, n1) in enumerate(NSL):
                nw = n1 - n0
                ps = psum.tile([128, 512], F32, tag="av", name="psv", bufs=3)
                for kc in range(KC):
                    nc.tensor.matmul(ps[:nw, :], xts[:, kc, n0:n1],
                                     wts[:, kc, 1024:1536],
                                     start=(kc == 0), stop=False)
                nc.tensor.matmul(ps[:nw, :], xone[:, n0:n1], wb[:, 1024:1536],
                                 start=False, stop=True)
                if si == 0:
                    vtk2 = pr.tile([2, HPC, 64], F32R, tag="vtk2", name="vtk2")
                    nc.scalar.activation(
                        vtk2[:],
                        ps[0:2, :].rearrange("p (h d) -> p h d", h=HPC),
                        AF.Identity)
                    nc.sync.dma_start(vtok0[:, :, 0:64], vtk2[0:1])
                    nc.sync.dma_start(vtok1[:, :, 0:64], vtk2[1:2])
                else:
                    nc.scalar.activation(
                        vz[:, si - 1, :, 0:64],
                        ps[:, :].rearrange("p (h d) -> p h d", h=HPC),
                        AF.Identity)

        if dbg:
            nc.sync.dma_start(dbg["d_qT"], qT[:, :, 0:N])
            nc.sync.dma_start(dbg["d_kT"], kT[:, :, 0:N])
            nc.sync.dma_start(dbg["d_vz"], vz[:])
        # ---------------- stage C: attention ----------------
        with tc.tile_pool(name="pc" + sfx, bufs=1) as pc, \
             tc.tile_pool(name="pe2" + sfx, bufs=2) as pe2:
            zt8 = pc.tile([8, N], MDT, name="zt8")
            e0t8 = pc.tile([8, N], MDT, name="e0t8")
            e1t8 = pc.tile([8, N], MDT, name="e1t8")
            r0t = pc.tile([8, N], F32, name="r0t")
            hr0 = pc.tile([8, N], MDT, name="hr0")
            hr1 = pc.tile([8, N], MDT, name="hr1")
            sgs = pc.tile([8, N], MDT, name="sgs")
            sgg0 = pc.tile([8, N], F32R, name="sgg0")
            sgg1 = pc.tile([8, N], F32R, name="sgg1")
            avsb = pc.tile([65, HPC, NP], MDT, name="avsb")
            # per-head gather rows packed along the free dim so the combine
            # matmul rhs / broadcast src sit at base partition 0

            e01p = pc.tile([2, HPC, NP], MDT, name="e01p")
            for h in range(HPC):
                j, po = h // 2, (h % 2) * 64
                for (q0, q1) in QCH:
                    qw = q1 - q0
                    qsl = slice(q0, q1)
                    et = pe2.tile([128, KC, QCW], MDT, tag="et", name="et", bufs=3)
                    for ktg in range(4):     # kt pairs
                        lg = psum.tile([128, 2, 512], F32, tag="lgt", name="lg", bufs=2)
                        for u in range(2):
                            kt = 2 * ktg + u
                            ksl = slice(2 + kt * 128, 2 + (kt + 1) * 128)
                            nc.tensor.matmul(
                                lg[:, u, :qw], kT[po:po + 64, j, ksl],
                                qT[po:po + 64, j, qsl], start=True, stop=True)
                        nc.scalar.activation(
                            et[:, 2 * ktg:2 * ktg + 2, :qw], lg[:, :, :qw],
                            AF.Exp, scale=SCALE)
                    av = psum.tile([65, 512], F32, tag="av", name="av", bufs=3)
                    for kt in range(KC):
                        nc.tensor.matmul(av[:, :qw], vz[:, kt, h, :],
                                         et[:, kt, :qw], start=(kt == 0),
                                         stop=(kt == KC - 1))
                    pstokh = psum.tile([2, 512], F32, tag="tok",
                                       name="pstokh")
                    nc.tensor.matmul(
                        pstokh[:, 0:qw], kT[po:po + 64, j, 0:2],
                        qT[po:po + 64, j, qsl], start=True, stop=True)
                    nc.scalar.activation(e01p[:, h, qsl], pstokh[:, 0:qw],
                                         AF.Exp, scale=SCALE)
                    # evacuate av (A rows + Z row) to SBUF in one op
                    nc.vector.tensor_copy(avsb[:, h, qsl], av[0:65, :qw])
                # head's token/denominator rows -> correction layout (1->1
                # partition DMAs; the BIR verifier rejects partition-count-
                # changing SBUF-SBUF DMAs)
                qa_ = nc.sync if h % 2 == 0 else nc.scalar
                qb_ = nc.scalar if h % 2 == 0 else nc.sync
                qa_.dma_start(zt8[h:h + 1, :], avsb[64:65, h, 0:N])
                qb_.dma_start(e0t8[h:h + 1, :], e01p[0:1, h, 0:N])
                qa_.dma_start(e1t8[h:h + 1, :], e01p[1:2, h, 0:N])
            # batched corrections (all heads, full width, [8, N] layout);
            # bf16 in/out keeps DVE in 2x/4x mode where possible
            nc.vector.tensor_tensor(out=r0t[:], in0=e0t8[:], in1=zt8[:], op=ADD)
            nc.vector.reciprocal(r0t[:], r0t[:])
            nc.vector.tensor_scalar_mul(hr0[:], r0t[:], 0.5)
            nc.vector.tensor_tensor(out=r0t[:], in0=e1t8[:], in1=zt8[:], op=ADD)
            nc.vector.reciprocal(r0t[:], r0t[:])
            nc.vector.tensor_scalar_mul(hr1[:], r0t[:], 0.5)
            nc.vector.tensor_tensor(out=sgs[:], in0=hr0[:], in1=hr1[:], op=ADD)
            nc.vector.tensor_tensor(out=sgg0[:], in0=e0t8[:], in1=hr0[:], op=MUL)
            nc.vector.tensor_tensor(out=sgg1[:], in0=e1t8[:], in1=hr1[:], op=MUL)
            # token-query fixups (cols 0,1): full r_t / e00*r_t, zero cross-t
            for t, (sgg, hrb, ebs) in enumerate(
                    [(sgg0, hr0, e0t8), (sgg1, hr1, e1t8)]):
                tc_ = slice(t, t + 1)
                nc.vector.tensor_tensor(out=sgs[:, tc_], in0=hrb[:, tc_],
                                        in1=hrb[:, tc_], op=ADD)
                nc.vector.tensor_tensor(out=sgg[:, tc_], in0=ebs[:, tc_],
                                        in1=hrb[:, tc_], op=MUL)
                nc.vector.tensor_scalar_mul(sgg[:, tc_], sgg[:, tc_], 2.0)
                oth = sgg1 if t == 0 else sgg0
                nc.vector.tensor_scalar_mul(oth[:, tc_], oth[:, tc_], 0.0)

            # final combine (h-outer): gather per-head rows into
            # free-dim-packed f32r tiles at base partition 0 (matmul rhs
            # constraint), broadcast the spatial scale, combine, then project.
            for h in range(HPC):
                j, po = h // 2, (h % 2) * 64
                sgsr = pe2.tile([1, N], MDT, tag="sgsr", name="sgsr", bufs=2)
                sgr0 = pe2.tile([1, N], F32R, tag="sgr0", name="sgr0", bufs=2)
                sgr1 = pe2.tile([1, N], F32R, tag="sgr1", name="sgr1", bufs=2)
                qa_ = nc.sync if h % 2 == 0 else nc.scalar
                qb_ = nc.scalar if h % 2 == 0 else nc.sync
                qa_.dma_start(sgsr[:], sgs[h:h + 1, :])
                qb_.dma_start(sgr0[:], sgg0[h:h + 1, :])
                qa_.dma_start(sgr1[:], sgg1[h:h + 1, :])
                bc = pe2.tile([64, N], MDT, tag="bc", name="bc", bufs=2)
                nc.gpsimd.partition_broadcast(bc[:], sgsr[0:1, :])
                for (q0, q1) in QCH:
                    qw = q1 - q0
                    qsl = slice(q0, q1)
                    vt = psum.tile([64, 512], F32, tag="av", name="vt", bufs=3)
                    nc.tensor.matmul(vt[:, :qw], vtok0[:, h, :64],
                                     sgr0[:, qsl], start=True, stop=False)
                    nc.tensor.matmul(vt[:, :qw], vtok1[:, h, :64],
                                     sgr1[:, qsl], start=False, stop=True)
                    tf = pe2.tile([64, 512], F32, tag="tf", name="tf")
                    nc.vector.tensor_tensor(out=tf[:, :qw],
                                            in0=avsb[0:64, h, qsl],
                                            in1=bc[:, qsl], op=MUL)
                    nc.vector.tensor_tensor(out=oT[po:po + 64, j, qsl],
                                            in0=vt[:, :qw], in1=tf[:, :qw],
                                            op=ADD)
            PSL = [(128 * i, 128 * (i + 1)) for i in range(8)] + [(1024, 1026)]
            for (n0, n1) in PSL:
                nw = n1 - n0
                for f in range(2):
                    ps = psum.tile([128, 512], F32, tag="lgt", name="psp",
                                   bufs=2)
                    for j in range(4):
                        nc.tensor.matmul(
                            ps[:nw, :], oT[:, j, n0:n1],
                            pjts[:, j, 512 * f:512 * (f + 1)],
                            start=(j == 0), stop=(j == 3))
                    ob = pe2.tile([128, 512], F32, tag="ob", name="ob",
                                  bufs=2)
                    nc.vector.tensor_copy(ob[:nw, :], ps[:nw, :])
                    nc.sync.dma_start(out[n0:n1, 512 * f:512 * (f + 1)],
                                      ob[:nw, :])

            if dbg:
                nc.sync.dma_start(dbg["d_avsb"], avsb[:, :, 0:N])
                nc.sync.dma_start(dbg["d_zb"][0:8], zt8[:])
                nc.sync.dma_start(dbg["d_zb"][8:16], e0t8[:])
                nc.sync.dma_start(dbg["d_sgs"], sgs[:])
                nc.sync.dma_start(dbg["d_sgg0"], sgg0[:])
        if dbg:
            nc.sync.dma_start(dbg["d_oT"], oT[:])


# ---------------- host side ----------------

_PERM = np.concatenate([np.arange(0, 64, 2), np.arange(1, 64, 2)])


def _host_prep(x, rope, qkv_w, q_bias, k_bias, v_bias, proj_w):
    """Build per-core input dicts."""
    import ml_dtypes
    bf = ml_dtypes.bfloat16
    x = np.asarray(x, np.float32)
    rope = np.asarray(rope, np.float32)
    qkv_w = np.asarray(qkv_w, np.float32)
    q_bias = np.asarray(q_bias, np.float32)
    k_bias = np.asarray(k_bias, np.float32)
    v_bias = np.asarray(v_bias, np.float32)
    proj_w = np.asarray(proj_w, np.float32)

    sin = rope[:, :D].T          # [64, S]
    cos = rope[:, D:].T
    cos64 = cos[_PERM]
    sinA64 = np.empty((64, S), np.float32)
    sinA64[0:32] = -sin[0::2]
    sinA64[32:64] = sin[1::2]
    cosP = np.vstack([cos64, cos64]).astype(np.float32)
    sinAF = np.vstack([sinA64, sinA64]).astype(np.float32)

    in_maps = []
    for core in range(NCORES):
        b, hh = core // 2, core % 2
        hs = hh * 512
        idx = np.concatenate([h * 64 + _PERM for h in range(HPC)]) + hs
        wq = qkv_w[0:C][idx]
        wk = qkv_w[C:2 * C][idx]
        wv = qkv_w[2 * C:3 * C][hs:hs + 512]
        W3 = np.concatenate([wq, wk, wv], 0)          # [1536, 1024]
        wTa = np.empty((1025, 1536), np.float32)
        wTa[0:1024] = W3.T
        wTa[1024] = np.concatenate(
            [q_bias[idx], k_bias[idx], v_bias[hs:hs + 512]])
        xTa = np.empty((1025, N), np.float32)
        xTa[0:1024] = x[b].T
        xTa[1024] = 1.0
        pjTa = np.ascontiguousarray(proj_w[:, hs:hs + 512].T)  # [512, 1024]
        in_maps.append({"xT": xTa, "wT": wTa, "pjT": pjTa,
                        "cosP": cosP, "sinAF": sinAF})
    return in_maps


def _get_runner():
    return _get_runner_rep(1)


def _get_runner_rep(repeat, serial=False):
    key = f"runner{repeat}s{int(serial)}"
    if key in _CACHE:
        return _CACHE[key]
    import jax
    from jax.sharding import Mesh, PartitionSpec
    from jax.experimental.shard_map import shard_map
    import concourse.mybir as mybir
    from concourse import bass2jax

    nc = _build_nc(repeat=repeat, serial=serial)
    bass2jax.install_neuronx_cc_hook()
    in_names, out_names, out_avals = [], [], []
    partition_name = (nc.partition_id_tensor.name
                      if nc.partition_id_tensor else None)
    for alloc in nc.m.functions[0].allocations:
        if not isinstance(alloc, mybir.MemoryLocationSet):
            continue
        name = alloc.memorylocations[0].name
        if alloc.kind == "ExternalInput":
            if name != partition_name:
                in_names.append(name)
        elif alloc.kind == "ExternalOutput":
            out_names.append(name)
            out_avals.append(jax.core.ShapedArray(
                tuple(alloc.tensor_shape), mybir.dt.np(alloc.dtype)))
    all_in = list(in_names) + list(out_names)
    if partition_name is not None:
        all_in.append(partition_name)

    def _body(*args):
        operands = list(args)
        if partition_name is not None:
            operands.append(bass2jax.partition_id_tensor())
        return tuple(bass2jax._bass_exec_p.bind(
            *operands, out_avals=tuple(out_avals), in_names=tuple(all_in),
            out_names=tuple(out_names), lowering_input_output_aliases=(),
            sim_require_finite=True, sim_require_nnan=True, nc=nc))

    mesh = Mesh(np.asarray(jax.devices()[:NCORES]), ("core",))
    nin = len(in_names)
    nout = len(out_names)
    fn = jax.jit(
        shard_map(_body, mesh=mesh,
                  in_specs=(PartitionSpec("core"),) * (nin + nout),
                  out_specs=(PartitionSpec("core"),) * nout,
                  check_rep=False),
        keep_unused=True)
    _CACHE[key] = (fn, mesh, in_names, out_names, out_avals)
    return _CACHE[key]


def kernel(x, rope, qkv_w, q_bias, k_bias, v_bias, proj_w, proj_b):
    import jax
    from jax.sharding import PartitionSpec
    fn, mesh, in_names, out_names, out_avals = _get_runner()
    in_maps = _host_prep(x, rope, qkv_w, q_bias, k_bias, v_bias, proj_w)
    sharding = jax.sharding.NamedSharding(mesh, PartitionSpec("core"))
    args = []
    for name in in_names:
        cat = np.concatenate([m[name] for m in in_maps], axis=0)
        args.append(jax.device_put(cat, sharding))
    for av in out_avals:
        z = np.zeros((NCORES * av.shape[0], *av.shape[1:]), av.dtype)
        args.append(jax.device_put(z, sharding))
    outs = fn(*args)
    parts = np.asarray(outs[out_names.index("out")]).reshape(
        NCORES, N, C)
    proj_b = np.asarray(proj_b, np.float32)
    res = np.empty((B, N, C), np.float32)
    for b in range(B):
        res[b] = parts[2 * b] + parts[2 * b + 1] + proj_b
    return res
